# revision 44
# baseline (speedup 1.0000x reference)
"""MirrorAttention Trainium2 kernel.

Data-parallel over batch B=8: one batch per NeuronCore (8 cores).
Each core computes, for its batch b:
    f_a = relu(bn(Wa @ x)),  f_v = relu(bn(Wv @ x_v)),  f_h = relu(bn(Wv @ x_h))
    A_d = softmax_rows(scale * f_qᵀ f_a)           (d in {v, h}, q in {f_v, f_h})
    g_d = Wg_d @ x + bg_d                          (kept transposed: [n, m])
    o_d = g_d @ A_d ; out_d = Wf_d @ o_d + bf_d + x
BN (eval, mean=0, var=1) is folded into the conv weights on the host.
Softmax normalization (1/rowsum) is folded into the gᵀ rows (contraction
index) instead of scaling the big A matrix.  Matmuls run in bf16 on the
PE; exp runs on ScalarE straight out of PSUM with fused row-sum
accumulation; the final "+ x" residual is added in fp32.
"""

import numpy as np
import ml_dtypes

import concourse.bass as bass
import concourse.mybir as mybir
import concourse.tile as tile
import bass_rust
from concourse.bass_utils import run_bass_kernel_spmd
from concourse.tile import add_dep_helper

B, C, H, W = 8, 512, 48, 48
MID = 128
N = H * W                     # 2304 tokens
NB = N // 128                 # 18 query blocks
CCH = C // 128                # 4 contraction chunks
SCALE = float(MID) ** -0.5
EPS = 1e-5
JTS = [(0, 512), (512, 512), (1024, 512), (1536, 512), (2048, 256)]
# S row split chosen so PE refill of one psum piece hides under the other
# piece's exp: [0,1536) = 3 matmuls, [1536,2304) = 2 matmuls.
NSPLIT = 1536
SJT0 = [(0, 512), (512, 512), (1024, 512)]
SJT1 = [(1536, 512), (2048, 256)]

F32 = mybir.dt.float32
BF16 = mybir.dt.bfloat16
BF = ml_dtypes.bfloat16
ADD = mybir.AluOpType.add
MAX = mybir.AluOpType.max


def _split_multi_waits(nc, max_waits=1):
    """walrus in this container rejects >1 sync-wait on CTRL-class
    instructions; hoist excess waits onto preceding NoOps."""
    for f in nc.m.functions:
        for bb in f.blocks:
            insts = list(bb.instructions)
            new, changed = [], False
            for inst in insts:
                si = inst.sync_info
                if si and si.on_wait and len(si.on_wait) > max_waits:
                    waits = list(si.on_wait)
                    k = 0
                    while len(waits) > max_waits:
                        chunk, waits = waits[:max_waits], waits[max_waits:]
                        nop = mybir.InstNoOp(
                            name=f"{inst.name}_waitsplit{k}", ins=[], outs=[]
                        )
                        nop.engine = inst.engine
                        nop.sync_info = bass_rust.SyncInfo(
                            on_wait=chunk, on_update=[]
                        )
                        new.append(nop)
                        k += 1
                    inst.sync_info = bass_rust.SyncInfo(
                        on_wait=waits, on_update=list(si.on_update)
                    )
                    changed = True
                new.append(inst)
            if changed:
                bb.instructions = new


def _build_nc():
    nc = bass.Bass()

    def din(name, shape, dt=F32):
        return nc.declare_dram_parameter(name, shape, dt, isOutput=False)

    xbf = din("xbf", [C, N], BF16)
    xhbf = din("xhbf", [C, N], BF16)
    xvbf = din("xvbf", [C, N], BF16)
    xf32 = din("xf32", [C, N], F32)

    # all weights packed host-side: 6 x [128, 512] bf16 slabs
    # (WaT, WvT, WgavT, WgahT interleaved [p, cch, m]; WfavT, WfahT [m, c])
    wpack = din("wpack", [128, 6 * C], BF16)
    # all biases packed: ba, bv, bfav(4), bfah(4), bgav-bcast(128),
    # bgah-bcast(128) = [128, 266] f32
    fpack = din("fpack", [128, 266], F32)

    oh = nc.declare_dram_parameter("oh", [C, N], F32, isOutput=True)
    ov = nc.declare_dram_parameter("ov", [C, N], F32, isOutput=True)

    with tile.TileContext(nc, pool_alloc_mode="queue") as tc:
        with (
            tc.tile_pool(name="consts", bufs=1) as consts,
            tc.tile_pool(name="fbuf", bufs=1) as fbuf,
            tc.tile_pool(name="gbuf", bufs=1) as gbuf,
        ):
            # --- stationary weights / biases: 2 packed DMAs ---
            wp = consts.tile([128, 6, CCH, MID], BF16, tag="wpack")
            nc.sync.dma_start(
                out=wp, in_=wpack[:].rearrange("p (w o m) -> p w o m", o=CCH, m=MID)
            )
            WaT_sb = wp[:, 0]
            WvT_sb = wp[:, 1]
            WgavT_sb = wp[:, 2]
            WgahT_sb = wp[:, 3]
            WfavT_sb = wp[:, 4].rearrange("p o m -> p (o m)")
            WfahT_sb = wp[:, 5].rearrange("p o m -> p (o m)")

            fp = consts.tile([128, 266], F32, tag="fpack")
            nc.sync.dma_start(out=fp, in_=fpack[:])
            ba_sb = fp[:, 0:1]
            bv_sb = fp[:, 1:2]
            bfav_sb = fp[:, 2 : 2 + CCH]
            bfah_sb = fp[:, 6 : 6 + CCH]
            bgav_sb = fp[:, 10:138]
            bgah_sb = fp[:, 138:266]

            # tiny dummy exp: fires immediately so the one-time ACT
            # exp-table load (~2.7us) overlaps the input-DMA ramp
            warm = consts.tile([128, 1], F32, tag="warmup")
            nc.vector.memset(warm, 0.0)
            nc.scalar.activation(
                out=warm, in_=warm,
                func=mybir.ActivationFunctionType.Exp, bias=0.0, scale=1.0,
            )

            # --- persistent activations ---
            f_a = fbuf.tile([128, N], BF16, tag="f_a")
            f_h = fbuf.tile([128, N], BF16, tag="f_h")
            gTav = gbuf.tile([128, NB, MID], BF16, tag="gTav")
            gTah = gbuf.tile([128, NB, MID], BF16, tag="gTah")

            def f_conv(psum_pool, W_sb, b_sb, src, dst, jts=JTS):
                # out[m, n] (m on partitions); bias+relu on DVE
                eps = []
                for j0, jw in jts:
                    pt = psum_pool.tile([128, 512], F32, tag=psum_pool.name)
                    for c in range(CCH):
                        nc.tensor.matmul(
                            pt[:, :jw],
                            lhsT=W_sb[:, c, :],
                            rhs=src[:, c, j0 : j0 + jw],
                            start=(c == 0),
                            stop=(c == CCH - 1),
                        )
                    eps.append(nc.vector.tensor_scalar(
                        out=dst[:, j0 : j0 + jw],
                        in0=pt[:, :jw],
                        scalar1=b_sb,
                        scalar2=0.0,
                        op0=ADD,
                        op1=MAX,
                    ))
                return eps

            def gt_fold(rinv, gT_sb):
                # fold 1/rowsum into gT rows, in place
                for blk in range(NB):
                    nc.vector.tensor_scalar_mul(
                        out=gT_sb[:, blk, :],
                        in0=gT_sb[:, blk, :],
                        scalar1=rinv[:, blk : blk + 1],
                    )

            def b1_scores(s2048, s256, f_q, A_sb, rs0, rs1, rinv,
                          fold_gT=None, blks=range(NB), finalize=True,
                          stream_apply=None):
                # scores + exp (+ row sums); exp reads PSUM directly.
                # fold_gT: emit the per-block rowsum/reciprocal/gT-fold
                # in-stream (requires gT bias-adds already emitted).
                for blk in blks:
                    q = f_q[:, blk * 128 : (blk + 1) * 128]
                    sa = s2048.tile([128, NSPLIT], F32, tag="sa")
                    for j0, jw in SJT0:
                        nc.tensor.matmul(
                            sa[:, j0 : j0 + jw],
                            lhsT=q,
                            rhs=f_a[:, j0 : j0 + jw],
                            start=True,
                            stop=True,
                        )
                    sb_ = s256.tile([128, N - NSPLIT], F32, tag="sb")
                    for j0, jw in SJT1:
                        nc.tensor.matmul(
                            sb_[:, j0 - NSPLIT : j0 - NSPLIT + jw],
                            lhsT=q,
                            rhs=f_a[:, j0 : j0 + jw],
                            start=True,
                            stop=True,
                        )
                    # scores are tiny (|scale*S| < 1): exp without
                    # max-subtraction is safe and exact
                    nc.scalar.activation(
                        out=A_sb[:, blk, 0:NSPLIT],
                        in_=sa,
                        func=mybir.ActivationFunctionType.Exp,
                        bias=0.0,
                        scale=SCALE,
                        accum_out=rs0[:, blk : blk + 1],
                    )
                    nc.scalar.activation(
                        out=A_sb[:, blk, NSPLIT:N],
                        in_=sb_,
                        func=mybir.ActivationFunctionType.Exp,
                        bias=0.0,
                        scale=SCALE,
                        accum_out=rs1[:, blk : blk + 1],
                    )
                    if fold_gT is not None:
                        b = slice(blk, blk + 1)
                        nc.vector.tensor_tensor(
                            out=rs0[:, b], in0=rs0[:, b], in1=rs1[:, b], op=ADD
                        )
                        nc.vector.reciprocal(out=rinv[:, b], in_=rs0[:, b])
                        nc.vector.tensor_scalar_mul(
                            out=fold_gT[:, blk, :],
                            in0=fold_gT[:, blk, :],
                            scalar1=rinv[:, b],
                        )
                    if stream_apply is not None:
                        park, gsc_sb, j0, jw = stream_apply
                        nc.tensor.matmul(
                            park[:, :jw],
                            lhsT=gsc_sb[:, blk, :],
                            rhs=A_sb[:, blk, j0 : j0 + jw],
                            start=(blk == 0),
                            stop=(blk == NB - 1),
                        )
                if fold_gT is not None or not finalize:
                    return None
                nc.vector.tensor_tensor(out=rs0, in0=rs0, in1=rs1, op=ADD)
                return nc.vector.reciprocal(out=rinv, in_=rs0)

            def b2_apply(opsum, cpsum, obf, outp, xfp, gsc, A_sb, Wf_sb,
                         bf_sb, out_dram, xt_after=None, out_eng=None,
                         jt0_park=None):
                if out_eng is None:
                    out_eng = nc.sync
                # o = gsc @ A, then out conv + bias + x (fp32)
                out_t = out_dram.rearrange("(o p) n -> p o n", p=128)
                x_t = xf32[:].rearrange("(o p) n -> p o n", p=128)
                for jt_i, (j0, jw) in enumerate(JTS):
                    if jt_i == 0 and jt0_park is not None:
                        ot = jt0_park  # accumulated in-stream during B1
                    else:
                        ot = opsum.tile([128, 512], F32, tag="opsum")
                        for blk in range(NB):
                            nc.tensor.matmul(
                                ot[:, :jw],
                                lhsT=gsc[:, blk, :],
                                rhs=A_sb[:, blk, j0 : j0 + jw],
                                start=(blk == 0),
                                stop=(blk == NB - 1),
                            )
                    o_bf = obf.tile([128, 512], BF16, tag="o_bf")
                    nc.vector.tensor_copy(out=o_bf[:, :jw], in_=ot[:, :jw])
                    # residual loads + stores batched per 2 channel-chunks
                    for half in range(2):
                        xt = xfp.tile([128, 2, 512], F32, tag="xt")
                        xd = nc.sync.dma_start(
                            out=xt[:, :, :jw],
                            in_=x_t[:, 2 * half : 2 * half + 2, j0 : j0 + jw],
                        )
                        if xt_after is not None:
                            add_dep_helper(
                                xd.ins, xt_after.ins, sync=True,
                                reason="x residual DMA after exp stream start",
                            )
                        outt = outp.tile([128, 2, 512], F32, tag="outt")
                        for ci in range(2):
                            co = 2 * half + ci
                            cp = cpsum.tile([128, 512], F32, tag="cpsum")
                            nc.tensor.matmul(
                                cp[:, :jw],
                                lhsT=Wf_sb[:, co * 128 : (co + 1) * 128],
                                rhs=o_bf[:, :jw],
                                start=True,
                                stop=True,
                            )
                            nc.vector.scalar_tensor_tensor(
                                out=outt[:, ci, :jw],
                                in0=cp[:, :jw],
                                scalar=bf_sb[:, co : co + 1],
                                in1=xt[:, ci, :jw],
                                op0=ADD,
                                op1=ADD,
                            )
                        out_eng.dma_start(
                            out=out_t[:, 2 * half : 2 * half + 2, j0 : j0 + jw],
                            in_=outt[:, :, :jw],
                        )

            # ---- long-lived stage-B pools first (pool release is LIFO:
            # pools closing mid-kernel must be created after these) ----
            with (
                tc.tile_pool(name="rpool", bufs=2) as rpool,
                tc.tile_pool(name="obf", bufs=1) as obf,
                tc.tile_pool(name="outp", bufs=2) as outp,
                tc.tile_pool(name="xfp", bufs=2) as xfp,
                tc.tile_pool(name="av", bufs=1) as av_pool,
                tc.tile_pool(name="opsum", bufs=2, space="PSUM") as opsum,
                tc.tile_pool(name="cpsum", bufs=1, space="PSUM") as cpsum,
            ):
                xpool_cm = tc.tile_pool(name="xpool", bufs=1)
                xpool = xpool_cm.__enter__()
                fv_cm = tc.tile_pool(name="fvkeep", bufs=1)
                fvkeep = fv_cm.__enter__()
                f_v = fvkeep.tile([128, N], BF16, tag="f_v")

                def load_x(pool, ap, tag, after=None):
                    # [C, N] -> [128, CCH, N]; one DMA per 128-channel chunk
                    t = pool.tile([128, CCH, N], BF16, tag=tag)
                    for c in range(CCH):
                        d = nc.sync.dma_start(
                            out=t[:, c, :],
                            in_=ap[c * 128 : (c + 1) * 128, :],
                        )
                        if after is not None:
                            add_dep_helper(
                                d.ins, after.ins, sync=True,
                                reason="input DMA ordering",
                            )
                    return t

                # ---- stage A-1 + B1(v), interleaved for an early exp
                # start: f_a fully + f_v's first tile, score blocks 0-3,
                # then the rest of f_v, then blocks 4-17.  Conv psum
                # borrows the (idle) B2 opsum slots. ----
                x_sb = load_x(xpool, xbf[:], "x")

                Av = av_pool.tile([128, NB, N], BF16, tag="Av")
                rs0v = rpool.tile([128, NB], F32, tag="rs0")
                rs1v = rpool.tile([128, NB], F32, tag="rs1")
                rinvv = rpool.tile([128, NB], F32, tag="rinv")

                s2048_cm = tc.tile_pool(name="s2048", bufs=1, space="PSUM")
                s2048 = s2048_cm.__enter__()
                s256_cm = tc.tile_pool(name="s256", bufs=1, space="PSUM")
                s256 = s256_cm.__enter__()

                with tc.tile_pool(name="xvin", bufs=1) as xvin:
                    xv_sb = load_x(xvin, xvbf[:], "xv")
                    f_conv(opsum, WaT_sb, ba_sb, x_sb, f_a)
                    fv_eps = f_conv(
                        opsum, WvT_sb, bv_sb, xv_sb, f_v, jts=JTS[:1]
                    )
                    b1_scores(s2048, s256, f_v, Av, rs0v, rs1v, rinvv,
                              blks=range(0, 4), finalize=False)
                    fv_eps += f_conv(
                        opsum, WvT_sb, bv_sb, xv_sb, f_v, jts=JTS[1:]
                    )

                rinvv_inst = b1_scores(
                    s2048, s256, f_v, Av, rs0v, rs1v, rinvv,
                    blks=range(4, NB),
                )

                with tc.tile_pool(name="xhin", bufs=1) as xhin:
                    xh_sb = load_x(xhin, xhbf[:], "xh", after=fv_eps[-1])
                    # filler convs borrow the idle B2 psum slots
                    f_conv(opsum, WvT_sb, bv_sb, xh_sb, f_h)
                    for W_sb, bb_sb, dst in (
                        (WgavT_sb, bgav_sb, gTav),
                        (WgahT_sb, bgah_sb, gTah),
                    ):
                        for blk in range(NB):
                            gp = cpsum.tile([128, MID], F32, tag="cpsum")
                            for c in range(CCH):
                                nc.tensor.matmul(
                                    gp,
                                    lhsT=x_sb[
                                        :, c, blk * 128 : (blk + 1) * 128
                                    ],
                                    rhs=W_sb[:, c, :],
                                    start=(c == 0),
                                    stop=(c == CCH - 1),
                                )
                            nc.vector.tensor_tensor(
                                out=dst[:, blk, :], in0=gp, in1=bb_sb, op=ADD
                            )

                gt_fold(rinvv, gTav)

                fv_cm.__exit__(None, None, None)     # f_v done
                xpool_cm.__exit__(None, None, None)  # x done

                ah_cm = tc.tile_pool(name="ah", bufs=1)
                ah_pool = ah_cm.__enter__()
                Ah = ah_pool.tile([128, NB, N], BF16, tag="Ah")
                rs0h = rpool.tile([128, NB], F32, tag="rs0")
                rs1h = rpool.tile([128, NB], F32, tag="rs1")
                rinvh = rpool.tile([128, NB], F32, tag="rinv")

                # B1(h); B2(v) emitted after = PE gap-filler during exps.
                # h's first apply tile accumulates in-stream in a parked
                # opsum slot so the tail starts with its conv immediately.
                oh_park = opsum.tile([128, 512], F32, tag="opsum")
                b1_scores(s2048, s256, f_h, Ah, rs0h, rs1h, rinvh,
                          fold_gT=gTah,
                          stream_apply=(oh_park, gTah, 0, 512))
                b2_apply(
                    opsum, cpsum, obf, outp, xfp,
                    gTav, Av, WfavT_sb, bfav_sb, ov, xt_after=rinvv_inst,
                )
                s256_cm.__exit__(None, None, None)
                s2048_cm.__exit__(None, None, None)

                # s pools closed: B2(h) gets deep psum pools for a fast tail
                with (
                    tc.tile_pool(name="opsumh", bufs=3, space="PSUM") as opsumh,
                    tc.tile_pool(name="cpsumh", bufs=2, space="PSUM") as cpsumh,
                ):
                    # h outputs go out on the ACT hwdge queue — ScalarE's
                    # instruction stream is past the exps by then, and the
                    # SP queue is busy with the xt residual loads
                    b2_apply(
                        opsumh, cpsumh, obf, outp, xfp,
                        gTah, Ah, WfahT_sb, bfah_sb, oh, out_eng=nc.scalar,
                        jt0_park=oh_park,
                    )
                ah_cm.__exit__(None, None, None)

    _split_multi_waits(nc)
    return nc


_NC = None


def _get_nc():
    global _NC
    if _NC is None:
        _NC = _build_nc()
    return _NC


def _fold_weights(Wa, ba, ga, ta, Wv, bv, gv, tv, Wgav, bgav, Wgah, bgah,
                  Wfav, bfav, Wfah, bfah):
    s_a = ga / np.sqrt(1.0 + EPS)
    s_v = gv / np.sqrt(1.0 + EPS)
    Wa_f = Wa * s_a[:, None]
    ba_f = ba * s_a + ta
    Wv_f = Wv * s_v[:, None]
    bv_f = bv * s_v + tv
    def wt_pre(W):  # [MID, C] weights -> W.T interleaved [128, CCH*MID]
        return W.T.reshape(CCH, 128, MID).transpose(1, 0, 2).reshape(128, CCH * MID)

    def col_pre(b):  # [C] -> [c % 128, c // 128]
        return b.reshape(CCH, 128).T

    wpack = np.concatenate(
        [wt_pre(Wa_f), wt_pre(Wv_f), wt_pre(Wgav), wt_pre(Wgah),
         Wfav.T, Wfah.T], axis=1
    )
    fpack = np.concatenate(
        [ba_f.reshape(MID, 1), bv_f.reshape(MID, 1),
         col_pre(bfav), col_pre(bfah),
         np.broadcast_to(bgav.reshape(1, MID), (128, MID)),
         np.broadcast_to(bgah.reshape(1, MID), (128, MID))], axis=1
    )
    return {
        "wpack": np.ascontiguousarray(wpack).astype(BF),
        "fpack": np.ascontiguousarray(fpack, dtype=np.float32),
    }


def kernel(x, x_h, x_v, Wa, ba, ga, ta, Wv, bv, gv, tv,
           Wgav, bgav, Wgah, bgah, Wfav, bfav, Wfah, bfah):
    x = np.asarray(x, dtype=np.float32)
    x_h = np.asarray(x_h, dtype=np.float32)
    x_v = np.asarray(x_v, dtype=np.float32)
    shared = _fold_weights(
        np.asarray(Wa, np.float32), np.asarray(ba, np.float32),
        np.asarray(ga, np.float32), np.asarray(ta, np.float32),
        np.asarray(Wv, np.float32), np.asarray(bv, np.float32),
        np.asarray(gv, np.float32), np.asarray(tv, np.float32),
        np.asarray(Wgav, np.float32), np.asarray(bgav, np.float32),
        np.asarray(Wgah, np.float32), np.asarray(bgah, np.float32),
        np.asarray(Wfav, np.float32), np.asarray(bfav, np.float32),
        np.asarray(Wfah, np.float32), np.asarray(bfah, np.float32),
    )

    in_maps = []
    for b in range(B):
        xb = np.ascontiguousarray(x[b].reshape(C, N))
        m = dict(shared)
        m["xbf"] = xb.astype(BF)
        m["xhbf"] = np.ascontiguousarray(x_h[b].reshape(C, N)).astype(BF)
        m["xvbf"] = np.ascontiguousarray(x_v[b].reshape(C, N)).astype(BF)
        m["xf32"] = xb
        in_maps.append(m)

    nc = _get_nc()
    res = run_bass_kernel_spmd(nc, in_maps, core_ids=list(range(B)))
    o_h = np.stack([res.results[b]["oh"].reshape(C, H, W) for b in range(B)])
    o_v = np.stack([res.results[b]["ov"].reshape(C, H, W) for b in range(B)])
    return (o_h, o_v)


# revision 45
# speedup vs baseline: 1.0026x; 1.0026x over previous
"""MirrorAttention Trainium2 kernel.

Data-parallel over batch B=8: one batch per NeuronCore (8 cores).
Each core computes, for its batch b:
    f_a = relu(bn(Wa @ x)),  f_v = relu(bn(Wv @ x_v)),  f_h = relu(bn(Wv @ x_h))
    A_d = softmax_rows(scale * f_qᵀ f_a)           (d in {v, h}, q in {f_v, f_h})
    g_d = Wg_d @ x + bg_d                          (kept transposed: [n, m])
    o_d = g_d @ A_d ; out_d = Wf_d @ o_d + bf_d + x
BN (eval, mean=0, var=1) is folded into the conv weights on the host.
Softmax normalization (1/rowsum) is folded into the gᵀ rows (contraction
index) instead of scaling the big A matrix.  Matmuls run in bf16 on the
PE; exp runs on ScalarE straight out of PSUM with fused row-sum
accumulation; the final "+ x" residual is added in fp32.
"""

import numpy as np
import ml_dtypes

import concourse.bass as bass
import concourse.mybir as mybir
import concourse.tile as tile
import bass_rust
from concourse.bass_utils import run_bass_kernel_spmd
from concourse.tile import add_dep_helper

B, C, H, W = 8, 512, 48, 48
MID = 128
N = H * W                     # 2304 tokens
NB = N // 128                 # 18 query blocks
CCH = C // 128                # 4 contraction chunks
SCALE = float(MID) ** -0.5
EPS = 1e-5
JTS = [(0, 512), (512, 512), (1024, 512), (1536, 512), (2048, 256)]
# S row split chosen so PE refill of one psum piece hides under the other
# piece's exp: [0,1536) = 3 matmuls, [1536,2304) = 2 matmuls.
NSPLIT = 1536
SJT0 = [(0, 512), (512, 512), (1024, 512)]
SJT1 = [(1536, 512), (2048, 256)]

F32 = mybir.dt.float32
BF16 = mybir.dt.bfloat16
BF = ml_dtypes.bfloat16
ADD = mybir.AluOpType.add
MAX = mybir.AluOpType.max


def _split_multi_waits(nc, max_waits=1):
    """walrus in this container rejects >1 sync-wait on CTRL-class
    instructions; hoist excess waits onto preceding NoOps."""
    for f in nc.m.functions:
        for bb in f.blocks:
            insts = list(bb.instructions)
            new, changed = [], False
            for inst in insts:
                si = inst.sync_info
                if si and si.on_wait and len(si.on_wait) > max_waits:
                    waits = list(si.on_wait)
                    k = 0
                    while len(waits) > max_waits:
                        chunk, waits = waits[:max_waits], waits[max_waits:]
                        nop = mybir.InstNoOp(
                            name=f"{inst.name}_waitsplit{k}", ins=[], outs=[]
                        )
                        nop.engine = inst.engine
                        nop.sync_info = bass_rust.SyncInfo(
                            on_wait=chunk, on_update=[]
                        )
                        new.append(nop)
                        k += 1
                    inst.sync_info = bass_rust.SyncInfo(
                        on_wait=waits, on_update=list(si.on_update)
                    )
                    changed = True
                new.append(inst)
            if changed:
                bb.instructions = new


def _build_nc():
    nc = bass.Bass()

    def din(name, shape, dt=F32):
        return nc.declare_dram_parameter(name, shape, dt, isOutput=False)

    xbf = din("xbf", [C, N], BF16)
    xhbf = din("xhbf", [C, N], BF16)
    xvbf = din("xvbf", [C, N], BF16)
    xf32 = din("xf32", [C, N], F32)

    # all weights packed host-side: 6 x [128, 512] bf16 slabs
    # (WaT, WvT, WgavT, WgahT interleaved [p, cch, m]; WfavT, WfahT [m, c])
    wpack = din("wpack", [128, 6 * C], BF16)
    # all biases packed: ba, bv, bfav(4), bfah(4), bgav-bcast(128),
    # bgah-bcast(128) = [128, 266] f32
    fpack = din("fpack", [128, 266], F32)

    oh = nc.declare_dram_parameter("oh", [C, N], F32, isOutput=True)
    ov = nc.declare_dram_parameter("ov", [C, N], F32, isOutput=True)

    with tile.TileContext(nc, pool_alloc_mode="queue") as tc:
        with (
            tc.tile_pool(name="consts", bufs=1) as consts,
            tc.tile_pool(name="fbuf", bufs=1) as fbuf,
            tc.tile_pool(name="gbuf", bufs=1) as gbuf,
        ):
            # --- stationary weights / biases: 2 packed DMAs ---
            wp = consts.tile([128, 6, CCH, MID], BF16, tag="wpack")
            nc.sync.dma_start(
                out=wp, in_=wpack[:].rearrange("p (w o m) -> p w o m", o=CCH, m=MID)
            )
            WaT_sb = wp[:, 0]
            WvT_sb = wp[:, 1]
            WgavT_sb = wp[:, 2]
            WgahT_sb = wp[:, 3]
            WfavT_sb = wp[:, 4].rearrange("p o m -> p (o m)")
            WfahT_sb = wp[:, 5].rearrange("p o m -> p (o m)")

            fp = consts.tile([128, 266], F32, tag="fpack")
            nc.sync.dma_start(out=fp, in_=fpack[:])
            ba_sb = fp[:, 0:1]
            bv_sb = fp[:, 1:2]
            bfav_sb = fp[:, 2 : 2 + CCH]
            bfah_sb = fp[:, 6 : 6 + CCH]
            bgav_sb = fp[:, 10:138]
            bgah_sb = fp[:, 138:266]

            # tiny dummy exp: fires immediately so the one-time ACT
            # exp-table load (~2.7us) overlaps the input-DMA ramp
            warm = consts.tile([128, 1], F32, tag="warmup")
            nc.vector.memset(warm, 0.0)
            nc.scalar.activation(
                out=warm, in_=warm,
                func=mybir.ActivationFunctionType.Exp, bias=0.0, scale=1.0,
            )

            # --- persistent activations ---
            f_a = fbuf.tile([128, N], BF16, tag="f_a")
            f_h = fbuf.tile([128, N], BF16, tag="f_h")
            gTav = gbuf.tile([128, NB, MID], BF16, tag="gTav")
            gTah = gbuf.tile([128, NB, MID], BF16, tag="gTah")

            def f_conv(psum_pool, W_sb, b_sb, src, dst, jts=JTS):
                # out[m, n] (m on partitions); bias+relu on DVE
                eps = []
                for j0, jw in jts:
                    pt = psum_pool.tile([128, 512], F32, tag=psum_pool.name)
                    for c in range(CCH):
                        nc.tensor.matmul(
                            pt[:, :jw],
                            lhsT=W_sb[:, c, :],
                            rhs=src[:, c, j0 : j0 + jw],
                            start=(c == 0),
                            stop=(c == CCH - 1),
                        )
                    eps.append(nc.vector.tensor_scalar(
                        out=dst[:, j0 : j0 + jw],
                        in0=pt[:, :jw],
                        scalar1=b_sb,
                        scalar2=0.0,
                        op0=ADD,
                        op1=MAX,
                    ))
                return eps

            def gt_fold(rinv, gT_sb):
                # fold 1/rowsum into gT rows, in place
                for blk in range(NB):
                    nc.vector.tensor_scalar_mul(
                        out=gT_sb[:, blk, :],
                        in0=gT_sb[:, blk, :],
                        scalar1=rinv[:, blk : blk + 1],
                    )

            def b1_scores(s2048, s256, f_q, A_sb, rs0, rs1, rinv,
                          fold_gT=None, blks=range(NB), finalize=True,
                          stream_apply=None):
                # scores + exp (+ row sums); exp reads PSUM directly.
                # fold_gT: emit the per-block rowsum/reciprocal/gT-fold
                # in-stream (requires gT bias-adds already emitted).
                for blk in blks:
                    q = f_q[:, blk * 128 : (blk + 1) * 128]
                    sa = s2048.tile([128, NSPLIT], F32, tag="sa")
                    for j0, jw in SJT0:
                        nc.tensor.matmul(
                            sa[:, j0 : j0 + jw],
                            lhsT=q,
                            rhs=f_a[:, j0 : j0 + jw],
                            start=True,
                            stop=True,
                        )
                    sb_ = s256.tile([128, N - NSPLIT], F32, tag="sb")
                    for j0, jw in SJT1:
                        nc.tensor.matmul(
                            sb_[:, j0 - NSPLIT : j0 - NSPLIT + jw],
                            lhsT=q,
                            rhs=f_a[:, j0 : j0 + jw],
                            start=True,
                            stop=True,
                        )
                    # scores are tiny (|scale*S| < 1): exp without
                    # max-subtraction is safe and exact
                    nc.scalar.activation(
                        out=A_sb[:, blk, 0:NSPLIT],
                        in_=sa,
                        func=mybir.ActivationFunctionType.Exp,
                        bias=0.0,
                        scale=SCALE,
                        accum_out=rs0[:, blk : blk + 1],
                    )
                    nc.scalar.activation(
                        out=A_sb[:, blk, NSPLIT:N],
                        in_=sb_,
                        func=mybir.ActivationFunctionType.Exp,
                        bias=0.0,
                        scale=SCALE,
                        accum_out=rs1[:, blk : blk + 1],
                    )
                    if fold_gT is not None:
                        b = slice(blk, blk + 1)
                        nc.vector.tensor_tensor(
                            out=rs0[:, b], in0=rs0[:, b], in1=rs1[:, b], op=ADD
                        )
                        nc.vector.reciprocal(out=rinv[:, b], in_=rs0[:, b])
                        nc.vector.tensor_scalar_mul(
                            out=fold_gT[:, blk, :],
                            in0=fold_gT[:, blk, :],
                            scalar1=rinv[:, b],
                        )
                    if stream_apply is not None:
                        park, gsc_sb, j0, jw = stream_apply
                        nc.tensor.matmul(
                            park[:, :jw],
                            lhsT=gsc_sb[:, blk, :],
                            rhs=A_sb[:, blk, j0 : j0 + jw],
                            start=(blk == 0),
                            stop=(blk == NB - 1),
                        )
                if fold_gT is not None or not finalize:
                    return None
                nc.vector.tensor_tensor(out=rs0, in0=rs0, in1=rs1, op=ADD)
                return nc.vector.reciprocal(out=rinv, in_=rs0)

            def b2_apply(opsum, cpsum, obf, outp, xfp, gsc, A_sb, Wf_sb,
                         bf_sb, out_dram, xt_after=None, out_eng=None,
                         jt0_park=None):
                if out_eng is None:
                    out_eng = nc.sync
                # o = gsc @ A, then out conv + bias + x (fp32)
                out_t = out_dram.rearrange("(o p) n -> p o n", p=128)
                x_t = xf32[:].rearrange("(o p) n -> p o n", p=128)
                for jt_i, (j0, jw) in enumerate(JTS):
                    if jt_i == 0 and jt0_park is not None:
                        ot = jt0_park  # accumulated in-stream during B1
                    else:
                        ot = opsum.tile([128, 512], F32, tag="opsum")
                        for blk in range(NB):
                            nc.tensor.matmul(
                                ot[:, :jw],
                                lhsT=gsc[:, blk, :],
                                rhs=A_sb[:, blk, j0 : j0 + jw],
                                start=(blk == 0),
                                stop=(blk == NB - 1),
                            )
                    o_bf = obf.tile([128, 512], BF16, tag="o_bf")
                    nc.vector.tensor_copy(out=o_bf[:, :jw], in_=ot[:, :jw])
                    # residual loads + stores batched per 2 channel-chunks
                    for half in range(2):
                        xt = xfp.tile([128, 2, 512], F32, tag="xt")
                        xd = nc.sync.dma_start(
                            out=xt[:, :, :jw],
                            in_=x_t[:, 2 * half : 2 * half + 2, j0 : j0 + jw],
                        )
                        if xt_after is not None:
                            add_dep_helper(
                                xd.ins, xt_after.ins, sync=True,
                                reason="x residual DMA after exp stream start",
                            )
                        outt = outp.tile([128, 2, 512], F32, tag="outt")
                        for ci in range(2):
                            co = 2 * half + ci
                            cp = cpsum.tile([128, 512], F32, tag="cpsum")
                            nc.tensor.matmul(
                                cp[:, :jw],
                                lhsT=Wf_sb[:, co * 128 : (co + 1) * 128],
                                rhs=o_bf[:, :jw],
                                start=True,
                                stop=True,
                            )
                            nc.vector.scalar_tensor_tensor(
                                out=outt[:, ci, :jw],
                                in0=cp[:, :jw],
                                scalar=bf_sb[:, co : co + 1],
                                in1=xt[:, ci, :jw],
                                op0=ADD,
                                op1=ADD,
                            )
                        out_eng.dma_start(
                            out=out_t[:, 2 * half : 2 * half + 2, j0 : j0 + jw],
                            in_=outt[:, :, :jw],
                        )

            # ---- long-lived stage-B pools first (pool release is LIFO:
            # pools closing mid-kernel must be created after these) ----
            with (
                tc.tile_pool(name="rpool", bufs=2) as rpool,
                tc.tile_pool(name="obf", bufs=1) as obf,
                tc.tile_pool(name="outp", bufs=2) as outp,
                tc.tile_pool(name="xfp", bufs=2) as xfp,
                tc.tile_pool(name="av", bufs=1) as av_pool,
                tc.tile_pool(name="opsum", bufs=2, space="PSUM") as opsum,
                tc.tile_pool(name="cpsum", bufs=1, space="PSUM") as cpsum,
            ):
                xpool_cm = tc.tile_pool(name="xpool", bufs=1)
                xpool = xpool_cm.__enter__()
                fv_cm = tc.tile_pool(name="fvkeep", bufs=1)
                fvkeep = fv_cm.__enter__()
                f_v = fvkeep.tile([128, N], BF16, tag="f_v")

                def load_x(pool, ap, tag, after=None):
                    # [C, N] -> [128, CCH, N]; one DMA per 128-channel chunk
                    t = pool.tile([128, CCH, N], BF16, tag=tag)
                    for c in range(CCH):
                        d = nc.sync.dma_start(
                            out=t[:, c, :],
                            in_=ap[c * 128 : (c + 1) * 128, :],
                        )
                        if after is not None:
                            add_dep_helper(
                                d.ins, after.ins, sync=True,
                                reason="input DMA ordering",
                            )
                    return t

                # ---- stage A-1 + B1(v), interleaved for an early exp
                # start: f_a fully + f_v's first tile, score blocks 0-3,
                # then the rest of f_v, then blocks 4-17.  Conv psum
                # borrows the (idle) B2 opsum slots. ----
                x_sb = load_x(xpool, xbf[:], "x")

                Av = av_pool.tile([128, NB, N], BF16, tag="Av")
                rs0v = rpool.tile([128, NB], F32, tag="rs0")
                rs1v = rpool.tile([128, NB], F32, tag="rs1")
                rinvv = rpool.tile([128, NB], F32, tag="rinv")

                s2048_cm = tc.tile_pool(name="s2048", bufs=1, space="PSUM")
                s2048 = s2048_cm.__enter__()
                s256_cm = tc.tile_pool(name="s256", bufs=1, space="PSUM")
                s256 = s256_cm.__enter__()

                with tc.tile_pool(name="xvin", bufs=1) as xvin:
                    xv_sb = load_x(xvin, xvbf[:], "xv")
                    f_conv(opsum, WaT_sb, ba_sb, x_sb, f_a)
                    fv_eps = f_conv(
                        opsum, WvT_sb, bv_sb, xv_sb, f_v, jts=JTS[:1]
                    )
                    b1_scores(s2048, s256, f_v, Av, rs0v, rs1v, rinvv,
                              blks=range(0, 4), finalize=False)
                    fv_eps += f_conv(
                        opsum, WvT_sb, bv_sb, xv_sb, f_v, jts=JTS[1:]
                    )

                rinvv_inst = b1_scores(
                    s2048, s256, f_v, Av, rs0v, rs1v, rinvv,
                    blks=range(4, NB),
                )

                with tc.tile_pool(name="xhin", bufs=1) as xhin:
                    xh_sb = load_x(xhin, xhbf[:], "xh", after=fv_eps[-1])
                    # filler convs borrow the idle B2 psum slots
                    f_conv(opsum, WvT_sb, bv_sb, xh_sb, f_h)
                    for W_sb, bb_sb, dst in (
                        (WgavT_sb, bgav_sb, gTav),
                        (WgahT_sb, bgah_sb, gTah),
                    ):
                        for blk in range(NB):
                            gp = cpsum.tile([128, MID], F32, tag="cpsum")
                            for c in range(CCH):
                                nc.tensor.matmul(
                                    gp,
                                    lhsT=x_sb[
                                        :, c, blk * 128 : (blk + 1) * 128
                                    ],
                                    rhs=W_sb[:, c, :],
                                    start=(c == 0),
                                    stop=(c == CCH - 1),
                                )
                            nc.vector.tensor_tensor(
                                out=dst[:, blk, :], in0=gp, in1=bb_sb, op=ADD
                            )

                gt_fold(rinvv, gTav)

                fv_cm.__exit__(None, None, None)     # f_v done
                xpool_cm.__exit__(None, None, None)  # x done

                ah_cm = tc.tile_pool(name="ah", bufs=1)
                ah_pool = ah_cm.__enter__()
                Ah = ah_pool.tile([128, NB, N], BF16, tag="Ah")
                rs0h = rpool.tile([128, NB], F32, tag="rs0")
                rs1h = rpool.tile([128, NB], F32, tag="rs1")
                rinvh = rpool.tile([128, NB], F32, tag="rinv")

                # B1(h); B2(v) emitted after = PE gap-filler during exps.
                # h's first apply tile accumulates in-stream in a parked
                # opsum slot so the tail starts with its conv immediately.
                oh_park = opsum.tile([128, 512], F32, tag="opsum")
                b1_scores(s2048, s256, f_h, Ah, rs0h, rs1h, rinvh,
                          fold_gT=gTah,
                          stream_apply=(oh_park, gTah, 0, 512))
                b2_apply(
                    opsum, cpsum, obf, outp, xfp,
                    gTav, Av, WfavT_sb, bfav_sb, ov, xt_after=rinvv_inst,
                )
                s256_cm.__exit__(None, None, None)
                s2048_cm.__exit__(None, None, None)

                # s pools closed: B2(h)'s apply rotates through the
                # already-live opsum slots (free as B2(v) drains, before the
                # freed s banks can re-allocate); convs get a fresh 2-deep
                # pool in the freed banks
                with (
                    tc.tile_pool(name="cpsumh", bufs=2, space="PSUM") as cpsumh,
                ):
                    # h outputs go out on the ACT hwdge queue — ScalarE's
                    # instruction stream is past the exps by then, and the
                    # SP queue is busy with the xt residual loads
                    b2_apply(
                        opsum, cpsumh, obf, outp, xfp,
                        gTah, Ah, WfahT_sb, bfah_sb, oh, out_eng=nc.scalar,
                        jt0_park=oh_park,
                    )
                ah_cm.__exit__(None, None, None)

    _split_multi_waits(nc)
    return nc


_NC = None


def _get_nc():
    global _NC
    if _NC is None:
        _NC = _build_nc()
    return _NC


def _fold_weights(Wa, ba, ga, ta, Wv, bv, gv, tv, Wgav, bgav, Wgah, bgah,
                  Wfav, bfav, Wfah, bfah):
    s_a = ga / np.sqrt(1.0 + EPS)
    s_v = gv / np.sqrt(1.0 + EPS)
    Wa_f = Wa * s_a[:, None]
    ba_f = ba * s_a + ta
    Wv_f = Wv * s_v[:, None]
    bv_f = bv * s_v + tv
    def wt_pre(W):  # [MID, C] weights -> W.T interleaved [128, CCH*MID]
        return W.T.reshape(CCH, 128, MID).transpose(1, 0, 2).reshape(128, CCH * MID)

    def col_pre(b):  # [C] -> [c % 128, c // 128]
        return b.reshape(CCH, 128).T

    wpack = np.concatenate(
        [wt_pre(Wa_f), wt_pre(Wv_f), wt_pre(Wgav), wt_pre(Wgah),
         Wfav.T, Wfah.T], axis=1
    )
    fpack = np.concatenate(
        [ba_f.reshape(MID, 1), bv_f.reshape(MID, 1),
         col_pre(bfav), col_pre(bfah),
         np.broadcast_to(bgav.reshape(1, MID), (128, MID)),
         np.broadcast_to(bgah.reshape(1, MID), (128, MID))], axis=1
    )
    return {
        "wpack": np.ascontiguousarray(wpack).astype(BF),
        "fpack": np.ascontiguousarray(fpack, dtype=np.float32),
    }


def kernel(x, x_h, x_v, Wa, ba, ga, ta, Wv, bv, gv, tv,
           Wgav, bgav, Wgah, bgah, Wfav, bfav, Wfah, bfah):
    x = np.asarray(x, dtype=np.float32)
    x_h = np.asarray(x_h, dtype=np.float32)
    x_v = np.asarray(x_v, dtype=np.float32)
    shared = _fold_weights(
        np.asarray(Wa, np.float32), np.asarray(ba, np.float32),
        np.asarray(ga, np.float32), np.asarray(ta, np.float32),
        np.asarray(Wv, np.float32), np.asarray(bv, np.float32),
        np.asarray(gv, np.float32), np.asarray(tv, np.float32),
        np.asarray(Wgav, np.float32), np.asarray(bgav, np.float32),
        np.asarray(Wgah, np.float32), np.asarray(bgah, np.float32),
        np.asarray(Wfav, np.float32), np.asarray(bfav, np.float32),
        np.asarray(Wfah, np.float32), np.asarray(bfah, np.float32),
    )

    in_maps = []
    for b in range(B):
        xb = np.ascontiguousarray(x[b].reshape(C, N))
        m = dict(shared)
        m["xbf"] = xb.astype(BF)
        m["xhbf"] = np.ascontiguousarray(x_h[b].reshape(C, N)).astype(BF)
        m["xvbf"] = np.ascontiguousarray(x_v[b].reshape(C, N)).astype(BF)
        m["xf32"] = xb
        in_maps.append(m)

    nc = _get_nc()
    res = run_bass_kernel_spmd(nc, in_maps, core_ids=list(range(B)))
    o_h = np.stack([res.results[b]["oh"].reshape(C, H, W) for b in range(B)])
    o_v = np.stack([res.results[b]["ov"].reshape(C, H, W) for b in range(B)])
    return (o_h, o_v)
